# revision 1
# baseline (speedup 1.0000x reference)
"""CondTransport kernel for 8x Trainium2 NeuronCores.

Math (per reference):
  x_mean = [x_mu, y_mean+y_var]                      [Nq, 64]
  x_var  = [x_mu, 0.01*flip(y_eta), y_mean+y_var]    [Nq, 96]
  Lam_m  = kXXmean_inv @ Z_mean                      [Nx, 32]
  Lam_v  = kXXvar_inv  @ Z_var                       [Nx, 32]
  K_m    = exp(-d2(X_mean, x_mean)/128);  z_m = K_m.T @ Lam_m
  K_v    = exp(-d2(X_var,  x_var )/128);  z_v = K_v.T @ Lam_v
  out    = y_mean + y_var + z_m + z_v                [Nq, 32]

Sharding: query dim across 8 cores (1024 rows each). The inverse Grams
are row-sharded for Lambda (core c computes Lam rows [1024c, 1024(c+1)))
and AllGathered on-device. Host prep is layout-only: transposes /
contiguous slicing / flip of input tensors.

RBF factorization: exp(-d2/128) = exp(S/64 - |X|^2/128) * exp(-|xq|^2/128)
with S = X @ xq.T.  The |X| term enters as the ACT per-partition bias,
the |xq| term is applied after the z matmul (valid because the reference
max(d2,0) clamp never binds for this data: d2 ~ 128 >> 0).

Dtypes: float32r (fp32 storage, 11-bit-mantissa matmul) for all big
matmul operands -> full PE rate; fp32 for norms/bias/final combine.
"""
import sys

sys.path.insert(0, "/opt/trn_rl_repo")

import numpy as np
from contextlib import ExitStack

import concourse.bacc as bacc
import concourse.bass as bass
import concourse.masks as masks
import concourse.mybir as mybir
import concourse.tile as tile
from concourse.bass_utils import run_bass_kernel_spmd

NX = 8192
NQ = 8192
DX = 32
DY = 32
DM = 64          # x_mean feature dim
DV = 96          # x_var feature dim
NCORES = 8
QLOC = NQ // NCORES           # 1024 queries per core
RLOC = NX // NCORES           # 1024 Lambda rows per core
NXT = NX // 128               # 64 x-tiles
QT = QLOC // 128              # 8 local q-tiles
QCH = QLOC // 512             # 2 local q-chunks of 512
INV_CHUNK = 256               # kappa rows per inv DMA chunk ([128, 2048] tile)
NCHUNK = NX // INV_CHUNK      # 32 chunks per matrix

F32 = mybir.dt.float32
F32R = mybir.dt.float32r
EXP = mybir.ActivationFunctionType.Exp
COPY = mybir.ActivationFunctionType.Copy

_CACHED_NC = None


def _build_nc(k_bufs=16, use_collective=True, do_a=True, do_b=True):
    nc = bacc.Bacc("TRN2", target_bir_lowering=False, debug=False,
                   num_devices=NCORES)

    # ---------------- I/O ----------------
    din = {}
    def inp(name, shape, dt=F32R):
        din[name] = nc.dram_tensor(name, list(shape), dt, kind="ExternalInput").ap()
        return din[name]

    invm = inp("invm", (NX, RLOC))        # kXXmean_inv.T[:, rows_c]
    invv = inp("invv", (NX, RLOC))        # kXXvar_inv.T[:, rows_c]
    XmT = inp("XmT", (DM, NX))            # X_mean.T
    XvT = inp("XvT", (DV, NX))            # X_var.T
    Zm = inp("Zm", (NX, DY))
    Zv = inp("Zv", (NX, DY))
    xmuT = inp("xmuT", (DX, QLOC))        # x_mu.T slice
    yefT = inp("yefT", (DY, QLOC))        # flip(y_eta).T slice (unscaled)
    ymT = inp("ymT", (DY, QLOC))
    yvT = inp("yvT", (DY, QLOC))
    Xm_nat = inp("Xm_nat", (NX, DM), F32)
    Xv_nat = inp("Xv_nat", (NX, DV), F32)
    xmu_nat = inp("xmu_nat", (QLOC, DX), F32)
    yef_nat = inp("yef_nat", (QLOC, DY), F32)
    ym_nat = inp("ym_nat", (QLOC, DY), F32)
    yv_nat = inp("yv_nat", (QLOC, DY), F32)

    out = nc.dram_tensor("out", [QLOC, DY], F32, kind="ExternalOutput").ap()

    # collective bounce buffers
    lam_in_m = nc.dram_tensor("lam_in_m", [RLOC, DY], F32, kind="Internal").ap()
    lam_in_v = nc.dram_tensor("lam_in_v", [RLOC, DY], F32, kind="Internal").ap()
    lam_out_m = nc.dram_tensor("lam_out_m", [NX, DY], F32, kind="Internal",
                               addr_space="Shared").ap()
    lam_out_v = nc.dram_tensor("lam_out_v", [NX, DY], F32, kind="Internal",
                               addr_space="Shared").ap()

    with tile.TileContext(nc) as tc, ExitStack() as ctx:
        P = lambda **kw: ctx.enter_context(tc.tile_pool(**kw))
        const_pool = P(name="const", bufs=1)
        inv_pool = P(name="inv", bufs=3)
        k_pool = P(name="ktile", bufs=k_bufs)
        psumS = P(name="psumS", bufs=2, space="PSUM")
        psumA = P(name="psumA", bufs=1, space="PSUM")
        psumZ = P(name="psumZ", bufs=1, space="PSUM")
        psumT = P(name="psumT", bufs=1, space="PSUM")
        work = P(name="work", bufs=2)
        stage_pool = P(name="stage", bufs=1)

        # ---------------- setup: small loads ----------------
        ident = const_pool.tile([128, 128], F32, tag="ident")
        masks.make_identity(nc, ident[:])

        XmT_sb = const_pool.tile([DM, NX], F32R, tag="XmT_sb")
        nc.sync.dma_start(XmT_sb[:], XmT)
        XvT_sb = const_pool.tile([DV, NX], F32R, tag="XvT_sb")
        nc.sync.dma_start(XvT_sb[:], XvT)

        Zm_sb = const_pool.tile([128, NXT * DY], F32R, tag="Zm_sb")
        nc.sync.dma_start(Zm_sb[:], Zm.rearrange("(t p) d -> p t d", p=128))
        Zv_sb = const_pool.tile([128, NXT * DY], F32R, tag="Zv_sb")
        nc.sync.dma_start(Zv_sb[:], Zv.rearrange("(t p) d -> p t d", p=128))

        # query slabs (transposed feature-major)
        xmT_sb = const_pool.tile([DM, QLOC], F32R, tag="xmT_sb")
        nc.sync.dma_start(xmT_sb[0:DX, :], xmuT)
        nc.sync.dma_start(xmT_sb[DX:DM, :], ymT)
        yv_scr = const_pool.tile([DM, QLOC], F32R, tag="yv_scr")
        nc.sync.dma_start(yv_scr[DX:DM, :], yvT)
        nc.vector.tensor_add(xmT_sb[DX:DM, :], xmT_sb[DX:DM, :], yv_scr[DX:DM, :])

        xvT_sb = const_pool.tile([DV, QLOC], F32R, tag="xvT_sb")
        nc.sync.dma_start(xvT_sb[0:DX, :], xmuT)
        nc.sync.dma_start(xvT_sb[DX:DM, :], yefT)
        nc.vector.tensor_scalar_mul(xvT_sb[DX:DM, :], xvT_sb[DX:DM, :], 0.01)
        nc.vector.tensor_copy(xvT_sb[DM:DV, :], xmT_sb[DX:DM, :])  # y_mean+y_var

        # ---------------- X row norms -> ACT bias (-|X|^2/128) ----------------
        Xn_m = const_pool.tile([128, NXT], F32, tag="Xn_m")
        Xn_v = const_pool.tile([128, NXT], F32, tag="Xn_v")
        for (nat, dfeat, dst) in ((Xm_nat, DM, Xn_m), (Xv_nat, DV, Xn_v)):
            for j in range(NXT):
                t = work.tile([128, DV], F32, tag="xnat")
                nc.sync.dma_start(t[:, 0:dfeat], nat[j * 128:(j + 1) * 128, :])
                sq = work.tile([128, DV], F32, tag="xsq")
                nc.vector.tensor_mul(sq[:, 0:dfeat], t[:, 0:dfeat], t[:, 0:dfeat])
                nc.vector.tensor_reduce(dst[:, j:j + 1], sq[:, 0:dfeat],
                                        mybir.AxisListType.X, mybir.AluOpType.add)
            nc.vector.tensor_scalar_mul(dst[:], dst[:], -1.0 / 128.0)

        # ---------------- query natural tiles -> q-norm scales ----------------
        # e_m = exp(-|x_mean_q|^2/128), e_v likewise, [128, QT] col-per-tile
        e_m = const_pool.tile([128, QT], F32, tag="e_m")
        e_v = const_pool.tile([128, QT], F32, tag="e_v")
        ymv_sb = const_pool.tile([128, QT * DY], F32, tag="ymv_sb")
        for j in range(QT):
            r0 = j * 128
            qn = work.tile([128, DV], F32, tag="qnat")
            nc.sync.dma_start(qn[:, 0:DX], xmu_nat[r0:r0 + 128, :])
            nc.sync.dma_start(qn[:, DX:DM], ym_nat[r0:r0 + 128, :])
            yvt = work.tile([128, DY], F32, tag="qyv")
            nc.sync.dma_start(yvt[:], yv_nat[r0:r0 + 128, :])
            # y_mean + y_var (also the final additive term)
            nc.vector.tensor_add(qn[:, DX:DM], qn[:, DX:DM], yvt[:])
            nc.vector.tensor_copy(ymv_sb[:, j * DY:(j + 1) * DY], qn[:, DX:DM])
            sq = work.tile([128, DV], F32, tag="qsq")
            nc.vector.tensor_mul(sq[:, 0:DM], qn[:, 0:DM], qn[:, 0:DM])
            nc.vector.tensor_reduce(e_m[:, j:j + 1], sq[:, 0:DM],
                                    mybir.AxisListType.X, mybir.AluOpType.add)
            # x_var natural: [x_mu, 0.01*yef, ymv]
            qv = work.tile([128, DV], F32, tag="qvnat")
            nc.vector.tensor_copy(qv[:, 0:DX], qn[:, 0:DX])
            yft = work.tile([128, DY], F32, tag="qyf")
            nc.sync.dma_start(yft[:], yef_nat[r0:r0 + 128, :])
            nc.vector.tensor_scalar_mul(qv[:, DX:DM], yft[:], 0.01)
            nc.vector.tensor_copy(qv[:, DM:DV], qn[:, DX:DM])
            nc.vector.tensor_mul(sq[:, 0:DV], qv[:, 0:DV], qv[:, 0:DV])
            nc.vector.tensor_reduce(e_v[:, j:j + 1], sq[:, 0:DV],
                                    mybir.AxisListType.X, mybir.AluOpType.add)
        nc.vector.tensor_scalar_mul(e_m[:], e_m[:], -1.0 / 128.0)
        nc.vector.tensor_scalar_mul(e_v[:], e_v[:], -1.0 / 128.0)
        nc.scalar.activation(e_m[:], e_m[:], EXP)
        nc.scalar.activation(e_v[:], e_v[:], EXP)

        # ---------------- stage A: Lam.T = Z.T @ invT, streamed ----------------
        def stage_a(inv_dram, Z_sb, lam_in):
            pa = [psumA.tile([DY, 512], F32, tag=f"pa{rc}", name=f"pa{rc}") for rc in range(2)]
            for c in range(NCHUNK):
                k0 = c * INV_CHUNK
                chunk = inv_pool.tile([128, 2 * RLOC], F32R, tag="invchunk")
                nc.sync.dma_start(
                    chunk[:],
                    inv_dram[k0:k0 + INV_CHUNK, :].rearrange(
                        "(two p) r -> p two r", p=128))
                for two in range(2):
                    kt = 2 * c + two
                    for rc in range(2):
                        nc.tensor.matmul(
                            pa[rc][:],
                            Z_sb[:, kt * DY:(kt + 1) * DY],
                            chunk[:, two * RLOC + rc * 512: two * RLOC + (rc + 1) * 512],
                            start=(kt == 0), stop=(kt == NXT - 1))
            # transpose LamT [32, 1024] -> natural [1024, 32] and ship to DRAM
            lamT = work.tile([DY, RLOC], F32, tag="lamT")
            for rc in range(2):
                nc.vector.tensor_copy(lamT[:, rc * 512:(rc + 1) * 512], pa[rc][:])
            lam_nat = work.tile([128, 8 * DY], F32, tag="lam_nat")
            for j in range(8):
                pt = psumT.tile([128, DY], F32, tag="pt")
                nc.tensor.transpose(pt[:], lamT[:, j * 128:(j + 1) * 128],
                                    ident[0:DY, 0:DY])
                nc.vector.tensor_copy(lam_nat[:, j * DY:(j + 1) * DY], pt[:])
            nc.scalar.dma_start(
                lam_in.rearrange("(t p) d -> p t d", p=128), lam_nat[:])

        if do_a:
            stage_a(invm, Zm_sb, lam_in_m)
            stage_a(invv, Zv_sb, lam_in_v)
        else:
            zero = work.tile([128, 8 * DY], F32, tag="lam_nat", name="zero_lam")
            nc.vector.tensor_scalar_mul(zero[:], Zm_sb[:, 0:8 * DY].bitcast(F32), 0.001)
            nc.scalar.dma_start(lam_in_m.rearrange("(t p) d -> p t d", p=128), zero[:])
            nc.scalar.dma_start(lam_in_v.rearrange("(t p) d -> p t d", p=128), zero[:])

        # gathers (issued in program order; deps keep them after stage A stores)
        def gather(lam_in, lam_out, lam_sb, stage_name):
            if use_collective:
                nc.gpsimd.collective_compute(
                    "AllGather", mybir.AluOpType.bypass,
                    replica_groups=[list(range(NCORES))],
                    ins=[lam_in.opt()], outs=[lam_out.opt()])
            else:
                for rep in range(NCORES):
                    nc.scalar.dma_start(
                        lam_out[rep * RLOC:(rep + 1) * RLOC, :], lam_in)
            stage = stage_pool.tile([128, NXT * DY], F32, tag="lam_stage",
                                    name=stage_name)
            nc.scalar.dma_start(
                stage[:], lam_out.rearrange("(t p) d -> p t d", p=128))
            nc.vector.tensor_copy(lam_sb[:], stage[:])  # round to f32r

        lam_m_sb = const_pool.tile([128, NXT * DY], F32R, tag="lam_m_sb")
        gather(lam_in_m, lam_out_m, lam_m_sb, "lam_stage_m")
        if not do_b:
            outz = const_pool.tile([128, QT * DY], F32, tag="outz")
            nc.vector.tensor_copy(outz[:], lam_m_sb[:, 0:QT * DY].bitcast(F32))
            for j in range(QT):
                nc.scalar.dma_start(out[j * 128:(j + 1) * 128, :],
                                    outz[:, j * DY:(j + 1) * DY])

        # ---------------- S/exp/z pipeline ----------------
        def phase(XT_sb, dfeat, xT_sb, Xn, lam_sb, zt_dst, e_scale, combine):
            pz = [psumZ.tile([DY, 512], F32, tag=f"pz{qc}", name=f"pz{qc}") for qc in range(2)]
            for j in range(NXT):
                for qc in range(QCH):
                    ps = psumS.tile([128, 512], F32, tag="ps")
                    nc.tensor.matmul(
                        ps[:],
                        XT_sb[:, j * 128:(j + 1) * 128],
                        xT_sb[:, qc * 512:(qc + 1) * 512],
                        start=True, stop=True)
                    kt = k_pool.tile([128, 512], F32R, tag="ktile")
                    nc.scalar.activation(kt[:], ps[:], EXP,
                                         bias=Xn[:, j:j + 1], scale=1.0 / 64.0)
                    nc.tensor.matmul(
                        pz[qc][:],
                        lam_sb[:, j * DY:(j + 1) * DY],
                        kt[:],
                        start=(j == 0), stop=(j == NXT - 1))
            zT = work.tile([DY, QLOC], F32, tag="zT")
            for qc in range(QCH):
                nc.vector.tensor_copy(zT[:, qc * 512:(qc + 1) * 512], pz[qc][:])
            for j in range(QT):
                pt = psumT.tile([128, DY], F32, tag="pt")
                nc.tensor.transpose(pt[:], zT[:, j * 128:(j + 1) * 128],
                                    ident[0:DY, 0:DY])
                combine(j, pt)

        zM_nat = const_pool.tile([128, QT * DY], F32, tag="zM_nat")

        def combine_mean(j, pt):
            nc.vector.tensor_scalar_mul(zM_nat[:, j * DY:(j + 1) * DY], pt[:],
                                        e_m[:, j:j + 1])

        if do_b:
            phase(XmT_sb, DM, xmT_sb, Xn_m, lam_m_sb, zM_nat, e_m, combine_mean)

        lam_v_sb = const_pool.tile([128, NXT * DY], F32R, tag="lam_v_sb")
        gather(lam_in_v, lam_out_v, lam_v_sb, "lam_stage_v")

        out_sb = const_pool.tile([128, QT * DY], F32, tag="out_sb")

        def combine_var(j, pt):
            sl = slice(j * DY, (j + 1) * DY)
            nc.vector.tensor_scalar_mul(out_sb[:, sl], pt[:], e_v[:, j:j + 1])
            nc.vector.tensor_add(out_sb[:, sl], out_sb[:, sl], zM_nat[:, sl])
            nc.vector.tensor_add(out_sb[:, sl], out_sb[:, sl], ymv_sb[:, sl])
            nc.scalar.dma_start(out[j * 128:(j + 1) * 128, :], out_sb[:, sl])

        if do_b:
            phase(XvT_sb, DV, xvT_sb, Xn_v, lam_v_sb, None, e_v, combine_var)

    nc.compile()
    return nc


import os
def get_nc():
    global _CACHED_NC
    if _CACHED_NC is None:
        _CACHED_NC = _build_nc(
            use_collective=os.environ.get("CTK_NO_COLLECTIVE", "") != "1",
            do_a=os.environ.get("CTK_SKIP_A", "") != "1",
            do_b=os.environ.get("CTK_SKIP_B", "") != "1")
    return _CACHED_NC


def _host_prep(x_mu, y_eta, y_mean, y_var, X_mean, X_var, Z_mean, Z_var,
               kXXmean_inv, kXXvar_inv):
    """Layout-only host prep: contiguous transposes / slices / flip."""
    C = np.ascontiguousarray
    invmT = C(kXXmean_inv.T)           # [NX, NX]
    invvT = C(kXXvar_inv.T)
    XmT = C(X_mean.T)
    XvT = C(X_var.T)
    yef = y_eta[::-1]                  # flip rows (unscaled)
    xmuT_f, yefT_f, ymT_f, yvT_f = C(x_mu.T), C(yef.T), C(y_mean.T), C(y_var.T)
    in_maps = []
    for c in range(NCORES):
        q = slice(c * QLOC, (c + 1) * QLOC)
        r = slice(c * RLOC, (c + 1) * RLOC)
        in_maps.append({
            "invm": C(invmT[:, r]),
            "invv": C(invvT[:, r]),
            "XmT": XmT, "XvT": XvT,
            "Zm": Z_mean, "Zv": Z_var,
            "xmuT": C(xmuT_f[:, q]), "yefT": C(yefT_f[:, q]),
            "ymT": C(ymT_f[:, q]), "yvT": C(yvT_f[:, q]),
            "Xm_nat": X_mean, "Xv_nat": X_var,
            "xmu_nat": C(x_mu[q]), "yef_nat": C(yef[q]),
            "ym_nat": C(y_mean[q]), "yv_nat": C(y_var[q]),
        })
    return in_maps


def kernel(x_mu, y_eta, y_mean, y_var, X_mean, X_var, Z_mean, Z_var,
           kXXmean_inv, kXXvar_inv, _trace=False, _tmpdir=None):
    nc = get_nc()
    in_maps = _host_prep(x_mu, y_eta, y_mean, y_var, X_mean, X_var,
                         Z_mean, Z_var, kXXmean_inv, kXXvar_inv)
    res = run_bass_kernel_spmd(nc, in_maps, core_ids=list(range(NCORES)),
                               trace=_trace, tmpdir=_tmpdir)
    out = np.concatenate([res.results[c]["out"] for c in range(NCORES)], axis=0)
    if _trace:
        kernel._last_results = res
    return out



# revision 28
# speedup vs baseline: 1.9357x; 1.9357x over previous
"""CondTransport kernel for 8x Trainium2 NeuronCores (v2, pipelined).

Math (per reference):
  x_mean = [x_mu, y_mean+y_var]                      [Nq, 64]
  x_var  = [x_mu, 0.01*flip(y_eta), y_mean+y_var]    [Nq, 96]
  Lam_m  = kXXmean_inv @ Z_mean                      [Nx, 32]
  Lam_v  = kXXvar_inv  @ Z_var                       [Nx, 32]
  K_m    = exp(-d2(X_mean, x_mean)/128);  z_m = K_m.T @ Lam_m
  K_v    = exp(-d2(X_var,  x_var )/128);  z_v = K_v.T @ Lam_v
  out    = y_mean + y_var + z_m + z_v                [Nq, 32]

Design notes (v2):
  - Queries sharded across 8 cores (1024 each); Lambda row-sharded with the
    row block split into S_SPLIT=4 sub-blocks, each AllGathered separately so
    Lambda tiles become available progressively while the RBF phases run.
  - All large matmul operands in bf16 (inv Grams cast on host): halves the
    dominant HBM stream and enables fast weight loads.
  - RBF factorization exp(-d2/128) = exp((S - |X|^2/2)/64) * exp(-|xq|^2/128)
    with the -|X|^2/2 term supplied as an EXTRA CONTRACTION ROW of the
    stationary operand (moving operand carries a ones-row), so the exp can
    run over [128, 4096] slabs with a single scale and no per-tile bias.
  - S tiles are DVE-copied from PSUM into [128, 4096] f32 slabs; one ACT exp
    per slab (amortizes the ~352-cycle ACT instruction overhead).
  - z and stage-A matmuls have 32-wide outputs: 4 are packed into the PE
    array concurrently via column tiling (tile_position), with a cross-
    partition DVE add at the end.
"""
import os
import sys

sys.path.insert(0, "/opt/trn_rl_repo")

import numpy as np
import ml_dtypes
from contextlib import ExitStack

import concourse.bacc as bacc
import concourse.masks as masks
import concourse.mybir as mybir
import concourse.tile as tile
from concourse.bass_utils import run_bass_kernel_spmd

NX = 8192
NQ = 8192
DX = 32
DY = 32
DM = 64          # x_mean feature dim
DV = 96          # x_var feature dim
NCORES = 8
QLOC = NQ // NCORES           # 1024 queries per core
RLOC = NX // NCORES           # 1024 Lambda rows per core
NXT = NX // 128               # 64 x-tiles
QT = QLOC // 128              # 8 local q-tiles

S_SPLIT = 4                   # Lambda sub-gathers per matrix
RSUB = RLOC // S_SPLIT        # 256 Lambda rows per core per sub-gather
ISUB = RSUB // 128            # 2 j-tiles contributed per core per sub-gather
NCH = 8                       # inv DMA chunks per sub-block
KTC = NXT // NCH              # 8 k-tiles per chunk
NST = 16                      # exp stage-tiles per phase (4 j-tiles each)
JPS = 4                       # j-tiles per stage
STW = JPS * QLOC              # stage width: 4096

F32 = mybir.dt.float32
BF16 = mybir.dt.bfloat16
EXP = mybir.ActivationFunctionType.Exp

_CACHED_NC = None

KT_BUFS = int(os.environ.get("CTK_KT_BUFS", "6"))
Z_LAG = int(os.environ.get("CTK_ZLAG", "2"))


def _jlist(phase_s0):
    """Phase j-tile consumption order: gather-availability order.

    Sub-gather s of this matrix yields j-tiles {8c + ISUB*s + i}.
    """
    out = []
    for s in range(S_SPLIT):
        for c in range(NCORES):
            for i in range(ISUB):
                out.append(8 * c + ISUB * s + i)
    return out


def _build_nc():
    nc = bacc.Bacc("TRN2", target_bir_lowering=False, debug=False,
                   num_devices=NCORES)

    # ---------------- I/O ----------------
    def inp(name, shape, dt=BF16):
        return nc.dram_tensor(name, list(shape), dt, kind="ExternalInput").ap()

    invm = inp("invm", (S_SPLIT, NCH, 128, KTC * RSUB))   # packed invT slabs
    invv = inp("invv", (S_SPLIT, NCH, 128, KTC * RSUB))
    XmT = inp("XmT", (DM, NX))            # X_mean.T
    XvT = inp("XvT", (DV, NX))            # X_var.T
    Zm = inp("Zm", (128, NXT * DY))       # packed (p, kt, d)
    Zv = inp("Zv", (128, NXT * DY))
    xmuT = inp("xmuT", (DX, QLOC))        # x_mu.T slice
    yefT = inp("yefT", (DY, QLOC))        # flip(y_eta).T slice (unscaled)
    ymT = inp("ymT", (DY, QLOC))
    yvT = inp("yvT", (DY, QLOC))
    qpk = inp("qpk", (128, 4 * QT * DY), F32)   # packed naturals (t, jq, d)
    Xmp = inp("Xmp", (128, NXT * DM))     # packed X_mean naturals (p, j, d)
    Xvp = inp("Xvp", (128, NXT * DV))

    out = nc.dram_tensor("out", [QLOC, DY], F32, kind="ExternalOutput").ap()
    xnr_d = {mat: nc.dram_tensor(f"xnr_{mat}", [1, NXT * 128], BF16,
                                 kind="Internal").ap() for mat in "mv"}

    # collective bounce buffers (per matrix x sub-block)
    lam_in = {}
    lam_out = {}
    for mat in "mv":
        for s in range(S_SPLIT):
            lam_in[mat, s] = nc.dram_tensor(
                f"lam_in_{mat}{s}", [RSUB, DY], F32, kind="Internal").ap()
            lam_out[mat, s] = nc.dram_tensor(
                f"lam_out_{mat}{s}", [NCORES * RSUB, DY], F32,
                kind="Internal", addr_space="Shared").ap()

    with tile.TileContext(nc) as tc, ExitStack() as ctx:
        P = lambda **kw: ctx.enter_context(tc.tile_pool(**kw))
        const = P(name="const", bufs=1)
        stgp = P(name="stgp", bufs=2)       # [128, 4096] f32 exp-input slabs
        xpk = P(name="xpk", bufs=2)         # X natural packs + squares
        ktp = P(name="ktp", bufs=KT_BUFS)   # [128, 4096] bf16 exp slabs
        invp = P(name="invp", bufs=3)       # inv chunks
        lstp = P(name="lstp", bufs=2)       # lambda gather stage-in
        work = P(name="work", bufs=1)
        psS = P(name="psS", bufs=2, space="PSUM")    # S matmul banks
        psZ = P(name="psZ", bufs=1, space="PSUM")    # z accumulators
        psA = P(name="psA", bufs=1, space="PSUM")    # stage-A accumulator
        psT = P(name="psT", bufs=2, space="PSUM")    # transposes

        ident = const.tile([128, 128], F32, tag="ident")
        masks.make_identity(nc, ident[:])

        # ---------------- setup loads (phase-m lead-in first) ----------------
        qpk_sb = const.tile([128, 4 * QT * DY], F32, tag="qpk_sb")
        nc.sync.dma_start(qpk_sb[:], qpk)

        XmT_sb = const.tile([DM + 1, NX], BF16, tag="XmT_sb")
        nc.sync.dma_start(XmT_sb[0:DM, :], XmT)
        Zm_sb = const.tile([128, NXT * DY], BF16, tag="Zm_sb")
        nc.sync.dma_start(Zm_sb[:], Zm)
        xmp_sb = xpk.tile([128, NXT * DV], BF16, tag="xpk", name="xmp_sb")
        nc.sync.dma_start(xmp_sb[:, 0:NXT * DM], Xmp)

        # ---------------- query-side assembly ----------------
        # DVE tensor ops need all operands at the same start partition, so
        # stage each transposed query block at its destination partition.
        xmT = const.tile([DM + 1, QLOC], BF16, tag="xmT")
        xvT = const.tile([DV + 1, QLOC], BF16, tag="xvT")
        ym_st = const.tile([DM, QLOC], BF16, tag="ym_st")
        yv_st = const.tile([DM, QLOC], BF16, tag="yv_st")
        yef_st = const.tile([DM, QLOC], BF16, tag="yef_st")
        nc.sync.dma_start(xmT[0:DX, :], xmuT)
        nc.sync.dma_start(xvT[0:DX, :], xmuT)
        nc.sync.dma_start(ym_st[DX:DM, :], ymT)
        nc.sync.dma_start(yv_st[DX:DM, :], yvT)
        nc.sync.dma_start(yef_st[DX:DM, :], yefT)
        nc.vector.tensor_add(xmT[DX:DM, :], ym_st[DX:DM, :], yv_st[DX:DM, :])
        nc.vector.memset(xmT[DM:DM + 1, :], 1.0)
        nc.vector.tensor_scalar_mul(xvT[DX:DM, :], yef_st[DX:DM, :], 0.01)
        # realign (ym+yv).T to partitions 64:96 via SBUF->SBUF DMA
        nc.scalar.dma_start(xvT[DM:DV, :], xmT[DX:DM, :])
        nc.vector.memset(xvT[DV:DV + 1, :], 1.0)

        # naturals: ymv (final additive term) + query norm scales
        ymv_sb = const.tile([128, QT * DY], F32, tag="ymv_sb")
        nc.vector.tensor_add(ymv_sb[:], qpk_sb[:, 2 * QT * DY:3 * QT * DY],
                             qpk_sb[:, 3 * QT * DY:4 * QT * DY])
        qsq = work.tile([128, 2 * QT * DY], F32, tag="qsq")
        nc.vector.tensor_mul(qsq[:, 0:QT * DY], qpk_sb[:, 0:QT * DY],
                             qpk_sb[:, 0:QT * DY])          # x_mu^2
        nc.vector.tensor_mul(qsq[:, QT * DY:], qpk_sb[:, QT * DY:2 * QT * DY],
                             qpk_sb[:, QT * DY:2 * QT * DY])  # yef^2
        ymvsq = work.tile([128, QT * DY], F32, tag="ymvsq")
        nc.vector.tensor_mul(ymvsq[:], ymv_sb[:], ymv_sb[:])
        r_xmu = work.tile([128, QT], F32, tag="r_xmu")
        r_yef = work.tile([128, QT], F32, tag="r_yef")
        r_ymv = work.tile([128, QT], F32, tag="r_ymv")
        RD = lambda dst, src: nc.vector.tensor_reduce(
            dst[:], src.rearrange("p (j d) -> p j d", d=DY),
            mybir.AxisListType.X, mybir.AluOpType.add)
        RD(r_xmu, qsq[:, 0:QT * DY])
        RD(r_yef, qsq[:, QT * DY:])
        RD(r_ymv, ymvsq[:])
        e_m = const.tile([128, QT], F32, tag="e_m")
        e_v = const.tile([128, QT], F32, tag="e_v")
        nc.vector.tensor_add(e_m[:], r_xmu[:], r_ymv[:])
        nc.vector.tensor_scalar_mul(r_yef[:], r_yef[:], 1.0e-4)
        nc.vector.tensor_add(e_v[:], e_m[:], r_yef[:])
        nc.scalar.activation(e_m[:], e_m[:], EXP, scale=-1.0 / 128.0)
        nc.scalar.activation(e_v[:], e_v[:], EXP, scale=-1.0 / 128.0)

        # ---------------- X norm rows (-|X|^2/2 into XT_sb row DM/DV) -------
        def x_norm_row(XT_sb, dfeat, xp_sb, mat):
            w = NXT * dfeat
            sq = xpk.tile([128, NXT * DV], BF16, tag="xpk", name="xsq")
            nc.vector.tensor_mul(sq[:, 0:w], xp_sb[:, 0:w], xp_sb[:, 0:w])
            xn = work.tile([128, NXT], F32, tag="xn")
            nc.vector.tensor_reduce(
                xn[:], sq[:, 0:w].rearrange("p (j d) -> p j d", d=dfeat),
                mybir.AxisListType.X, mybir.AluOpType.add)
            nc.vector.tensor_scalar_mul(xn[:], xn[:], -0.5)
            pt = psA.tile([128, 256], F32, tag="pa", name="xn_t")
            nc.tensor.transpose(pt[0:NXT, 0:128], xn[:], ident[:])
            xnT = work.tile([NXT, 128], BF16, tag="xnT")
            nc.vector.tensor_copy(xnT[:], pt[0:NXT, 0:128])
            # flatten [64(j), 128(p)] row-major onto the single XT norm row,
            # bouncing through DRAM (single-partition scatter APs are illegal)
            nc.scalar.dma_start(
                xnr_d[mat].rearrange("o (j p) -> j (o p)", j=NXT), xnT[:])
            nc.scalar.dma_start(XT_sb[dfeat:dfeat + 1, :], xnr_d[mat])

        x_norm_row(XmT_sb, DM, xmp_sb, "m")

        # ---------------- stage A: Lambda sub-blocks + sub-gathers ----------
        def stage_a_sub(inv_dram, Z_sb, mat, s):
            pa = psA.tile([128, RSUB], F32, tag="pa", name=f"pa_{mat}{s}")
            for q in range(NCH):
                chunk = invp.tile([128, KTC * RSUB], BF16, tag="invchunk")
                nc.sync.dma_start(chunk[:], inv_dram[s, q])
                for k8 in range(KTC):
                    kt = q * KTC + k8
                    g = kt % 4
                    nc.tensor.matmul(
                        pa[32 * g:32 * (g + 1), :],
                        Z_sb[:, kt * DY:(kt + 1) * DY],
                        chunk[:, k8 * RSUB:(k8 + 1) * RSUB],
                        start=(kt < 4), stop=(kt >= NXT - 4),
                        tile_position=(0, 32 * g))
            # transpose col-group partials so the group sum becomes a legal
            # same-base free-dim add: pa [4g*32d, r] -> T [r, 4g*32d]
            paS = work.tile([128, RSUB], F32, tag="paS")
            nc.vector.tensor_copy(paS[:], pa[:])
            lam_nat = work.tile([128, ISUB * DY], F32, tag="lam_nat")
            for i in range(ISUB):
                pt = psT.tile([128, 128], F32, tag="pt")
                nc.tensor.transpose(pt[:], paS[:, i * 128:(i + 1) * 128],
                                    ident[:])
                ptS = work.tile([128, 128], F32, tag="ptS")
                nc.vector.tensor_copy(ptS[:], pt[:])
                t0 = work.tile([128, DY], F32, tag="lam_t0")
                t1 = work.tile([128, DY], F32, tag="lam_t1")
                nc.vector.tensor_add(t0[:], ptS[:, 0:32], ptS[:, 32:64])
                nc.vector.tensor_add(t1[:], ptS[:, 64:96], ptS[:, 96:128])
                nc.vector.tensor_add(lam_nat[:, i * DY:(i + 1) * DY],
                                     t0[:], t1[:])
            nc.scalar.dma_start(
                lam_in[mat, s].rearrange("(i p) d -> p i d", p=128),
                lam_nat[:].rearrange("p (i d) -> p i d", d=DY))
            nc.gpsimd.collective_compute(
                "AllGather", mybir.AluOpType.bypass,
                replica_groups=[list(range(NCORES))],
                ins=[lam_in[mat, s].opt()], outs=[lam_out[mat, s].opt()])

        def lam_stage_in(lam_sb, mat, s):
            lst = lstp.tile([128, NCORES * ISUB * DY], F32, tag="lst")
            nc.scalar.dma_start(
                lst[:].rearrange("p (c i d) -> p c i d", c=NCORES, i=ISUB),
                lam_out[mat, s].rearrange("(c i p) d -> p c i d",
                                          i=ISUB, p=128))
            # scatter to lam_sb columns: j = 8c + ISUB*s + i
            dst = lam_sb[:].rearrange("p (c r) -> p c r", c=NCORES)[
                :, :, ISUB * DY * s:ISUB * DY * (s + 1)]
            nc.vector.tensor_copy(
                dst, lst[:].rearrange("p (c r) -> p c r", c=NCORES))

        lam_m_sb = const.tile([128, NXT * DY], BF16, tag="lam_m_sb")
        lam_v_sb = const.tile([128, NXT * DY], BF16, tag="lam_v_sb")
        XvT_sb = const.tile([DV + 1, NX], BF16, tag="XvT_sb")
        Zv_sb = const.tile([128, NXT * DY], BF16, tag="Zv_sb")

        # ---------------- RBF phase machinery ----------------
        def phase_mk(XT_sb, dfeat, xT_sb, lam_sb, combine, name):
            """Returns (emit_stage(st), emit_z(st), tail()) closures."""
            jl = _jlist(0)
            kts = [None] * NST
            pz = [psZ.tile([128, 512], F32, tag=f"pz{qc}", name=f"pz{qc}_{name}")
                  for qc in range(2)]

            def emit_stage(st):
                jset = jl[JPS * st:JPS * (st + 1)]
                stg = stgp.tile([128, STW], F32, tag="stg", name=f"stg_{name}{st}")
                for jj, j in enumerate(jset):
                    for qc in range(2):
                        ps = psS.tile([128, 512], F32, tag="ps")
                        nc.tensor.matmul(
                            ps[:],
                            XT_sb[0:dfeat + 1, j * 128:(j + 1) * 128],
                            xT_sb[0:dfeat + 1, qc * 512:(qc + 1) * 512],
                            start=True, stop=True)
                        nc.vector.tensor_copy(
                            stg[:, (2 * jj + qc) * 512:(2 * jj + qc + 1) * 512],
                            ps[:])
                kt = ktp.tile([128, STW], BF16, tag="kt")
                nc.scalar.activation(kt[:], stg[:], EXP, scale=1.0 / 64.0)
                kts[st] = kt

            def emit_z(st):
                jset = jl[JPS * st:JPS * (st + 1)]
                kt = kts[st]
                for qc in range(2):
                    for g, j in enumerate(jset):
                        nc.tensor.matmul(
                            pz[qc][32 * g:32 * (g + 1), :],
                            lam_sb[:, j * DY:(j + 1) * DY],
                            kt[:, (2 * g + qc) * 512:(2 * g + qc + 1) * 512],
                            start=(st == 0), stop=(st == NST - 1),
                            tile_position=(0, 32 * g))

            def tail():
                # pz [4g*32d, q] -> transpose 128-col slices -> [q, 4g*32d],
                # then sum groups along the free dim (natural q layout).
                for qc in range(2):
                    pzS = work.tile([128, 512], F32, tag="pzS")
                    nc.vector.tensor_copy(pzS[:], pz[qc][:])
                    for qq in range(4):
                        jq = 4 * qc + qq
                        pt = psT.tile([128, 128], F32, tag="pt")
                        nc.tensor.transpose(
                            pt[:], pzS[:, qq * 128:(qq + 1) * 128], ident[:])
                        ptS = work.tile([128, 128], F32, tag="ptS")
                        nc.vector.tensor_copy(ptS[:], pt[:])
                        t0 = work.tile([128, DY], F32, tag="z_t0")
                        t1 = work.tile([128, DY], F32, tag="z_t1")
                        nc.vector.tensor_add(t0[:], ptS[:, 0:32], ptS[:, 32:64])
                        nc.vector.tensor_add(t1[:], ptS[:, 64:96], ptS[:, 96:128])
                        combine(jq, t0, t1)

            return emit_stage, emit_z, tail

        zM_nat = const.tile([128, QT * DY], F32, tag="zM_nat")

        def combine_mean(jq, t0, t1):
            sl = slice(jq * DY, (jq + 1) * DY)
            nc.vector.tensor_add(zM_nat[:, sl], t0[:], t1[:])
            nc.vector.tensor_scalar_mul(zM_nat[:, sl], zM_nat[:, sl],
                                        e_m[:, jq:jq + 1])

        out_sb = const.tile([128, QT * DY], F32, tag="out_sb")

        def combine_var(jq, t0, t1):
            sl = slice(jq * DY, (jq + 1) * DY)
            nc.vector.tensor_add(out_sb[:, sl], t0[:], t1[:])
            nc.vector.tensor_scalar_mul(out_sb[:, sl], out_sb[:, sl],
                                        e_v[:, jq:jq + 1])
            nc.vector.tensor_add(out_sb[:, sl], out_sb[:, sl], zM_nat[:, sl])
            nc.vector.tensor_add(out_sb[:, sl], out_sb[:, sl], ymv_sb[:, sl])

        ph_m = phase_mk(XmT_sb, DM, xmT, lam_m_sb, combine_mean, "m")
        ph_v = phase_mk(XvT_sb, DV, xvT, lam_v_sb, combine_var, "v")

        # ---------------- emission schedule ----------------
        # stage A m: first two sub-blocks up front
        stage_a_sub(invm, Zm_sb, "m", 0)
        lam_stage_in(lam_m_sb, "m", 0)
        stage_a_sub(invm, Zm_sb, "m", 1)
        lam_stage_in(lam_m_sb, "m", 1)

        # var-side setup loads (deprioritized after inv m s0/s1)
        nc.gpsimd.dma_start(XvT_sb[0:DV, :], XvT)
        nc.gpsimd.dma_start(Zv_sb[:], Zv)

        def phase_emit(ph, st):
            emit_stage, emit_z, _ = ph
            emit_stage(st)
            if st - Z_LAG >= 0:
                emit_z(st - Z_LAG)

        # phase m stages 0..7 with stage A m s2/s3 interleaved
        for st in range(0, 4):
            phase_emit(ph_m, st)
        stage_a_sub(invm, Zm_sb, "m", 2)
        lam_stage_in(lam_m_sb, "m", 2)
        for st in range(4, 8):
            phase_emit(ph_m, st)
        stage_a_sub(invm, Zm_sb, "m", 3)
        lam_stage_in(lam_m_sb, "m", 3)

        # var X packs + norm row before phase v needs them
        xvp_sb = xpk.tile([128, NXT * DV], BF16, tag="xpk", name="xvp_sb")
        nc.gpsimd.dma_start(xvp_sb[:], Xvp)
        x_norm_row(XvT_sb, DV, xvp_sb, "v")

        for st in range(8, 12):
            phase_emit(ph_m, st)
        stage_a_sub(invv, Zv_sb, "v", 0)
        lam_stage_in(lam_v_sb, "v", 0)
        for st in range(12, 16):
            phase_emit(ph_m, st)
        stage_a_sub(invv, Zv_sb, "v", 1)
        lam_stage_in(lam_v_sb, "v", 1)
        # trailing z of phase m
        for st in range(NST - Z_LAG, NST):
            ph_m[1](st)
        ph_m[2]()  # tail m

        for st in range(0, 4):
            phase_emit(ph_v, st)
        stage_a_sub(invv, Zv_sb, "v", 2)
        lam_stage_in(lam_v_sb, "v", 2)
        for st in range(4, 8):
            phase_emit(ph_v, st)
        stage_a_sub(invv, Zv_sb, "v", 3)
        lam_stage_in(lam_v_sb, "v", 3)
        for st in range(8, 16):
            phase_emit(ph_v, st)
        for st in range(NST - Z_LAG, NST):
            ph_v[1](st)
        ph_v[2]()  # tail v (writes out_sb)

        nc.scalar.dma_start(out.rearrange("(t p) d -> p t d", p=128),
                            out_sb[:].rearrange("p (t d) -> p t d", d=DY))

    nc.compile()
    return nc


def get_nc():
    global _CACHED_NC
    if _CACHED_NC is None:
        _CACHED_NC = _build_nc()
    return _CACHED_NC


def _host_prep(x_mu, y_eta, y_mean, y_var, X_mean, X_var, Z_mean, Z_var,
               kXXmean_inv, kXXvar_inv):
    """Host prep: transposes / slicing / packing / bf16 casts only."""
    BF = ml_dtypes.bfloat16
    C = np.ascontiguousarray

    def pack_inv(inv):
        # per-core [S_SPLIT, NCH, 128, KTC*RSUB] slabs of inv.T in bf16
        invT = C(inv.T).astype(BF)
        packs = []
        for c in range(NCORES):
            A = invT[:, c * RLOC:(c + 1) * RLOC]
            A = A.reshape(NXT, 128, S_SPLIT, RSUB).transpose(2, 0, 1, 3)
            A = A.reshape(S_SPLIT, NCH, KTC, 128, RSUB).transpose(0, 1, 3, 2, 4)
            packs.append(C(A.reshape(S_SPLIT, NCH, 128, KTC * RSUB)))
        return packs

    invm_p = pack_inv(kXXmean_inv)
    invv_p = pack_inv(kXXvar_inv)

    XmT = C(X_mean.T).astype(BF)
    XvT = C(X_var.T).astype(BF)
    Zm = C(Z_mean.reshape(NXT, 128, DY).transpose(1, 0, 2).reshape(128, -1)
           ).astype(BF)
    Zv = C(Z_var.reshape(NXT, 128, DY).transpose(1, 0, 2).reshape(128, -1)
           ).astype(BF)
    Xmp = C(X_mean.reshape(NXT, 128, DM).transpose(1, 0, 2).reshape(128, -1)
            ).astype(BF)
    Xvp = C(X_var.reshape(NXT, 128, DV).transpose(1, 0, 2).reshape(128, -1)
            ).astype(BF)

    yef = y_eta[::-1]

    in_maps = []
    for c in range(NCORES):
        q = slice(c * QLOC, (c + 1) * QLOC)
        qpk = np.stack([x_mu[q], yef[q], y_mean[q], y_var[q]])  # [4,1024,32]
        qpk = qpk.reshape(4, QT, 128, DY).transpose(2, 0, 1, 3)
        in_maps.append({
            "invm": invm_p[c], "invv": invv_p[c],
            "XmT": XmT, "XvT": XvT, "Zm": Zm, "Zv": Zv,
            "xmuT": C(x_mu[q].T).astype(BF),
            "yefT": C(yef[q].T).astype(BF),
            "ymT": C(y_mean[q].T).astype(BF),
            "yvT": C(y_var[q].T).astype(BF),
            "qpk": C(qpk.reshape(128, -1)).astype(np.float32),
            "Xmp": Xmp, "Xvp": Xvp,
        })
    return in_maps


def kernel(x_mu, y_eta, y_mean, y_var, X_mean, X_var, Z_mean, Z_var,
           kXXmean_inv, kXXvar_inv, _trace=False, _tmpdir=None):
    nc = get_nc()
    in_maps = _host_prep(x_mu, y_eta, y_mean, y_var, X_mean, X_var,
                         Z_mean, Z_var, kXXmean_inv, kXXvar_inv)
    res = run_bass_kernel_spmd(nc, in_maps, core_ids=list(range(NCORES)),
                               trace=_trace, tmpdir=_tmpdir)
    out = np.concatenate([res.results[c]["out"] for c in range(NCORES)], axis=0)
    if _trace:
        kernel._last_results = res
    return out


# revision 35
# speedup vs baseline: 2.1965x; 1.1347x over previous
"""CondTransport kernel for 8x Trainium2 NeuronCores (v2, pipelined).

Math (per reference):
  x_mean = [x_mu, y_mean+y_var]                      [Nq, 64]
  x_var  = [x_mu, 0.01*flip(y_eta), y_mean+y_var]    [Nq, 96]
  Lam_m  = kXXmean_inv @ Z_mean                      [Nx, 32]
  Lam_v  = kXXvar_inv  @ Z_var                       [Nx, 32]
  K_m    = exp(-d2(X_mean, x_mean)/128);  z_m = K_m.T @ Lam_m
  K_v    = exp(-d2(X_var,  x_var )/128);  z_v = K_v.T @ Lam_v
  out    = y_mean + y_var + z_m + z_v                [Nq, 32]

Design notes (v2):
  - Queries sharded across 8 cores (1024 each); Lambda row-sharded with the
    row block split into S_SPLIT=4 sub-blocks, each AllGathered separately so
    Lambda tiles become available progressively while the RBF phases run.
  - All large matmul operands in bf16 (inv Grams cast on host): halves the
    dominant HBM stream and enables fast weight loads.
  - RBF factorization exp(-d2/128) = exp((S - |X|^2/2)/64) * exp(-|xq|^2/128)
    with the -|X|^2/2 term supplied as an EXTRA CONTRACTION ROW of the
    stationary operand (moving operand carries a ones-row), so the exp can
    run over [128, 4096] slabs with a single scale and no per-tile bias.
  - S tiles are DVE-copied from PSUM into [128, 4096] f32 slabs; one ACT exp
    per slab (amortizes the ~352-cycle ACT instruction overhead).
  - z and stage-A matmuls have 32-wide outputs: 4 are packed into the PE
    array concurrently via column tiling (tile_position), with a cross-
    partition DVE add at the end.
"""
import os
import sys

sys.path.insert(0, "/opt/trn_rl_repo")

import numpy as np
import ml_dtypes
from contextlib import ExitStack

import concourse.bacc as bacc
import concourse.masks as masks
import concourse.mybir as mybir
import concourse.tile as tile
from concourse.bass_utils import run_bass_kernel_spmd

NX = 8192
NQ = 8192
DX = 32
DY = 32
DM = 64          # x_mean feature dim
DV = 96          # x_var feature dim
NCORES = 8
QLOC = NQ // NCORES           # 1024 queries per core
RLOC = NX // NCORES           # 1024 Lambda rows per core
NXT = NX // 128               # 64 x-tiles
QT = QLOC // 128              # 8 local q-tiles

S_SPLIT = 4                   # Lambda sub-gathers per matrix
RSUB = RLOC // S_SPLIT        # 256 Lambda rows per core per sub-gather
ISUB = RSUB // 128            # 2 j-tiles contributed per core per sub-gather
NCH = 8                       # inv DMA chunks per sub-block
KTC = NXT // NCH              # 8 k-tiles per chunk
NST = 16                      # exp stage-tiles per phase (4 j-tiles each)
JPS = 4                       # j-tiles per stage
STW = JPS * QLOC              # stage width: 4096

F32 = mybir.dt.float32
BF16 = mybir.dt.bfloat16
EXP = mybir.ActivationFunctionType.Exp

_CACHED_NC = None

KT_BUFS = int(os.environ.get("CTK_KT_BUFS", "28"))
Z_LAG = int(os.environ.get("CTK_ZLAG", "2"))


def _jlist(phase_s0):
    """Phase j-tile consumption order: gather-availability order.

    Sub-gather s of this matrix yields j-tiles {8c + ISUB*s + i}.
    """
    out = []
    for s in range(S_SPLIT):
        for c in range(NCORES):
            for i in range(ISUB):
                out.append(8 * c + ISUB * s + i)
    return out


def _build_nc():
    nc = bacc.Bacc("TRN2", target_bir_lowering=False, debug=False,
                   num_devices=NCORES)

    # ---------------- I/O ----------------
    def inp(name, shape, dt=BF16):
        return nc.dram_tensor(name, list(shape), dt, kind="ExternalInput").ap()

    invm = inp("invm", (S_SPLIT, NCH, 128, KTC * RSUB))   # packed invT slabs
    invv = inp("invv", (S_SPLIT, NCH, 128, KTC * RSUB))
    XmT = inp("XmT", (DM, NX))            # X_mean.T
    XvT = inp("XvT", (DV, NX))            # X_var.T
    Zm = inp("Zm", (128, NXT * DY))       # packed (p, kt, d)
    Zv = inp("Zv", (128, NXT * DY))
    xmuT = inp("xmuT", (DX, QLOC))        # x_mu.T slice
    yefT = inp("yefT", (DY, QLOC))        # flip(y_eta).T slice (unscaled)
    ymT = inp("ymT", (DY, QLOC))
    yvT = inp("yvT", (DY, QLOC))
    qpk = inp("qpk", (128, 4 * QT * DY), F32)   # packed naturals (t, jq, d)
    Xmp = inp("Xmp", (128, NXT * DM))     # packed X_mean naturals (p, j, d)
    Xvp = inp("Xvp", (128, NXT * DV))

    out = nc.dram_tensor("out", [QLOC, DY], F32, kind="ExternalOutput").ap()
    xnr_d = {mat: nc.dram_tensor(f"xnr_{mat}", [1, NXT * 128], BF16,
                                 kind="Internal").ap() for mat in "mv"}

    # collective bounce buffers (per matrix x sub-block)
    lam_in = {}
    lam_out = {}
    for mat in "mv":
        for s in range(S_SPLIT):
            lam_in[mat, s] = nc.dram_tensor(
                f"lam_in_{mat}{s}", [RSUB, DY], F32, kind="Internal").ap()
            lam_out[mat, s] = nc.dram_tensor(
                f"lam_out_{mat}{s}", [NCORES * RSUB, DY], F32,
                kind="Internal", addr_space="Shared").ap()

    with tile.TileContext(nc) as tc, ExitStack() as ctx:
        P = lambda **kw: ctx.enter_context(tc.tile_pool(**kw))
        const = P(name="const", bufs=1)
        xpk = P(name="xpk", bufs=2)         # X natural packs + squares
        ktp = P(name="ktp", bufs=KT_BUFS)   # [128, 1024] bf16 exp tiles
        invp = P(name="invp", bufs=3)       # inv chunks
        lstp = P(name="lstp", bufs=2)       # lambda gather stage-in
        work = P(name="work", bufs=1)
        psS = P(name="psS", bufs=2, space="PSUM")    # S matmul 2-bank pairs
        psZ = P(name="psZ", bufs=1, space="PSUM")    # z accumulators
        psA = P(name="psA", bufs=1, space="PSUM")    # stage-A accumulator
        psT = P(name="psT", bufs=1, space="PSUM")    # transposes

        ident = const.tile([128, 128], F32, tag="ident")
        masks.make_identity(nc, ident[:])

        # ------- setup loads: X-norm chain first (it gates all S matmuls) ---
        xmp_sb = xpk.tile([128, NXT * DV], BF16, tag="xpk", name="xmp_sb")
        nc.sync.dma_start(xmp_sb[:, 0:NXT * DM], Xmp)
        XmT_sb = const.tile([DM + 1, NX], BF16, tag="XmT_sb")
        nc.sync.dma_start(XmT_sb[0:DM, :], XmT)

        # ---------------- X norm rows (-|X|^2/2 into XT_sb row DM/DV) -------
        def x_norm_row(XT_sb, dfeat, xp_sb, mat):
            w = NXT * dfeat
            sq = xpk.tile([128, NXT * DV], BF16, tag="xpk", name="xsq")
            nc.vector.tensor_mul(sq[:, 0:w], xp_sb[:, 0:w], xp_sb[:, 0:w])
            xn = work.tile([128, NXT], F32, tag="xn")
            nc.vector.tensor_reduce(
                xn[:], sq[:, 0:w].rearrange("p (j d) -> p j d", d=dfeat),
                mybir.AxisListType.X, mybir.AluOpType.add)
            nc.vector.tensor_scalar_mul(xn[:], xn[:], -0.5)
            pt = psT.tile([128, 128], F32, tag="pt", name=f"xn_t{mat}")
            nc.tensor.transpose(pt[0:NXT, 0:128], xn[:], ident[:])
            xnT = work.tile([NXT, 128], BF16, tag="xnT")
            nc.vector.tensor_copy(xnT[:], pt[0:NXT, 0:128])
            # flatten [64(j), 128(p)] row-major onto the single XT norm row,
            # bouncing through DRAM (single-partition scatter APs are illegal)
            nc.scalar.dma_start(
                xnr_d[mat].rearrange("o (j p) -> j (o p)", j=NXT), xnT[:])
            nc.scalar.dma_start(XT_sb[dfeat:dfeat + 1, :], xnr_d[mat])

        x_norm_row(XmT_sb, DM, xmp_sb, "m")

        qpk_sb = const.tile([128, 4 * QT * DY], F32, tag="qpk_sb")
        nc.sync.dma_start(qpk_sb[:], qpk)
        Zm_sb = const.tile([128, NXT * DY], BF16, tag="Zm_sb")
        nc.sync.dma_start(Zm_sb[:], Zm)

        # ---------------- query-side assembly ----------------
        # DVE tensor ops need all operands at the same start partition, so
        # stage each transposed query block at its destination partition.
        xmT = const.tile([DM + 1, QLOC], BF16, tag="xmT")
        xvT = const.tile([DV + 1, QLOC], BF16, tag="xvT")
        ym_st = const.tile([DM, QLOC], BF16, tag="ym_st")
        yv_st = const.tile([DM, QLOC], BF16, tag="yv_st")
        yef_st = const.tile([DM, QLOC], BF16, tag="yef_st")
        nc.sync.dma_start(xmT[0:DX, :], xmuT)
        nc.sync.dma_start(xvT[0:DX, :], xmuT)
        nc.sync.dma_start(ym_st[DX:DM, :], ymT)
        nc.sync.dma_start(yv_st[DX:DM, :], yvT)
        nc.sync.dma_start(yef_st[DX:DM, :], yefT)
        nc.vector.tensor_add(xmT[DX:DM, :], ym_st[DX:DM, :], yv_st[DX:DM, :])
        nc.vector.memset(xmT[DM:DM + 1, :], 1.0)
        nc.vector.tensor_scalar_mul(xvT[DX:DM, :], yef_st[DX:DM, :], 0.01)
        # realign (ym+yv).T to partitions 64:96 via SBUF->SBUF DMA
        nc.scalar.dma_start(xvT[DM:DV, :], xmT[DX:DM, :])
        nc.vector.memset(xvT[DV:DV + 1, :], 1.0)

        # naturals: ymv (final additive term) + query norm scales
        ymv_sb = const.tile([128, QT * DY], F32, tag="ymv_sb")
        nc.vector.tensor_add(ymv_sb[:], qpk_sb[:, 2 * QT * DY:3 * QT * DY],
                             qpk_sb[:, 3 * QT * DY:4 * QT * DY])
        qsq = work.tile([128, 2 * QT * DY], F32, tag="qsq")
        nc.vector.tensor_mul(qsq[:, 0:QT * DY], qpk_sb[:, 0:QT * DY],
                             qpk_sb[:, 0:QT * DY])          # x_mu^2
        nc.vector.tensor_mul(qsq[:, QT * DY:], qpk_sb[:, QT * DY:2 * QT * DY],
                             qpk_sb[:, QT * DY:2 * QT * DY])  # yef^2
        ymvsq = work.tile([128, QT * DY], F32, tag="ymvsq")
        nc.vector.tensor_mul(ymvsq[:], ymv_sb[:], ymv_sb[:])
        r_xmu = work.tile([128, QT], F32, tag="r_xmu")
        r_yef = work.tile([128, QT], F32, tag="r_yef")
        r_ymv = work.tile([128, QT], F32, tag="r_ymv")
        RD = lambda dst, src: nc.vector.tensor_reduce(
            dst[:], src.rearrange("p (j d) -> p j d", d=DY),
            mybir.AxisListType.X, mybir.AluOpType.add)
        RD(r_xmu, qsq[:, 0:QT * DY])
        RD(r_yef, qsq[:, QT * DY:])
        RD(r_ymv, ymvsq[:])
        e_m = const.tile([128, QT], F32, tag="e_m")
        e_v = const.tile([128, QT], F32, tag="e_v")
        nc.vector.tensor_add(e_m[:], r_xmu[:], r_ymv[:])
        nc.vector.tensor_scalar_mul(r_yef[:], r_yef[:], 1.0e-4)
        nc.vector.tensor_add(e_v[:], e_m[:], r_yef[:])
        nc.scalar.activation(e_m[:], e_m[:], EXP, scale=-1.0 / 128.0)
        nc.scalar.activation(e_v[:], e_v[:], EXP, scale=-1.0 / 128.0)

        # ---------------- stage A: Lambda sub-blocks + sub-gathers ----------
        def stage_a_sub(inv_dram, Z_sb, mat, s):
            pa = psA.tile([128, RSUB], F32, tag="pa", name=f"pa_{mat}{s}")
            for q in range(NCH):
                chunk = invp.tile([128, KTC * RSUB], BF16, tag="invchunk")
                nc.sync.dma_start(chunk[:], inv_dram[s, q])
                for k8 in range(KTC):
                    kt = q * KTC + k8
                    g = kt % 4
                    nc.tensor.matmul(
                        pa[32 * g:32 * (g + 1), :],
                        Z_sb[:, kt * DY:(kt + 1) * DY],
                        chunk[:, k8 * RSUB:(k8 + 1) * RSUB],
                        start=(kt < 4), stop=(kt >= NXT - 4),
                        tile_position=(0, 32 * g))
            # transpose col-group partials so the group sum becomes a legal
            # same-base free-dim add: pa [4g*32d, r] -> T [r, 4g*32d]
            paS = work.tile([128, RSUB], F32, tag="paS")
            nc.vector.tensor_copy(paS[:], pa[:])
            lam_nat = work.tile([128, ISUB * DY], F32, tag="lam_nat")
            for i in range(ISUB):
                pt = psT.tile([128, 128], F32, tag="pt")
                nc.tensor.transpose(pt[:], paS[:, i * 128:(i + 1) * 128],
                                    ident[:])
                t0 = work.tile([128, 2 * DY], F32, tag="lam_t0")
                nc.vector.tensor_copy(t0[:], pt[:, 0:64])
                nc.vector.tensor_add(t0[:], t0[:], pt[:, 64:128])
                nc.vector.tensor_add(lam_nat[:, i * DY:(i + 1) * DY],
                                     t0[:, 0:DY], t0[:, DY:2 * DY])
            nc.scalar.dma_start(
                lam_in[mat, s].rearrange("(i p) d -> p i d", p=128),
                lam_nat[:].rearrange("p (i d) -> p i d", d=DY))
            nc.gpsimd.collective_compute(
                "AllGather", mybir.AluOpType.bypass,
                replica_groups=[list(range(NCORES))],
                ins=[lam_in[mat, s].opt()], outs=[lam_out[mat, s].opt()])

        def lam_stage_in(lam_sb, mat, s):
            lst = lstp.tile([128, NCORES * ISUB * DY], F32, tag="lst")
            nc.scalar.dma_start(
                lst[:].rearrange("p (c i d) -> p c i d", c=NCORES, i=ISUB),
                lam_out[mat, s].rearrange("(c i p) d -> p c i d",
                                          i=ISUB, p=128))
            # scatter to lam_sb columns: j = 8c + ISUB*s + i
            dst = lam_sb[:].rearrange("p (c r) -> p c r", c=NCORES)[
                :, :, ISUB * DY * s:ISUB * DY * (s + 1)]
            nc.vector.tensor_copy(
                dst, lst[:].rearrange("p (c r) -> p c r", c=NCORES))

        lam_m_sb = const.tile([128, NXT * DY], BF16, tag="lam_m_sb")
        lam_v_sb = const.tile([128, NXT * DY], BF16, tag="lam_v_sb")
        XvT_sb = const.tile([DV + 1, NX], BF16, tag="XvT_sb")
        Zv_sb = const.tile([128, NXT * DY], BF16, tag="Zv_sb")

        # ---------------- RBF phase machinery ----------------
        def phase_mk(XT_sb, dfeat, xT_sb, lam_sb, combine, name):
            """Returns (emit_stage(st), emit_z(st), tail()) closures."""
            jl = _jlist(0)
            kts = {}
            pz = [psZ.tile([128, 512], F32, tag=f"pz{qc}", name=f"pz{qc}_{name}")
                  for qc in range(2)]

            def emit_stage(st):
                jset = jl[JPS * st:JPS * (st + 1)]
                for jj, j in enumerate(jset):
                    ps = psS.tile([128, 1024], F32, tag="ps")
                    for qc in range(2):
                        nc.tensor.matmul(
                            ps[:, qc * 512:(qc + 1) * 512],
                            XT_sb[0:dfeat + 1, j * 128:(j + 1) * 128],
                            xT_sb[0:dfeat + 1, qc * 512:(qc + 1) * 512],
                            start=True, stop=True)
                    kt = ktp.tile([128, QLOC], BF16, tag="kt")
                    nc.scalar.activation(kt[:], ps[:], EXP, scale=1.0 / 64.0)
                    kts[st, jj] = kt

            def emit_z(st):
                jset = jl[JPS * st:JPS * (st + 1)]
                for qc in range(2):
                    for g, j in enumerate(jset):
                        nc.tensor.matmul(
                            pz[qc][32 * g:32 * (g + 1), :],
                            lam_sb[:, j * DY:(j + 1) * DY],
                            kts[st, g][:, qc * 512:(qc + 1) * 512],
                            start=(st == 0), stop=(st == NST - 1),
                            tile_position=(0, 32 * g))

            def tail():
                # pz [4g*32d, q] -> transpose 128-col slices -> [q, 4g*32d],
                # then sum groups along the free dim (natural q layout).
                for qc in range(2):
                    pzS = work.tile([128, 512], F32, tag="pzS")
                    nc.vector.tensor_copy(pzS[:], pz[qc][:])
                    for qq in range(4):
                        jq = 4 * qc + qq
                        pt = psT.tile([128, 128], F32, tag="pt")
                        nc.tensor.transpose(
                            pt[:], pzS[:, qq * 128:(qq + 1) * 128], ident[:])
                        t0 = work.tile([128, 2 * DY], F32, tag="z_t0")
                        nc.vector.tensor_copy(t0[:], pt[:, 0:64])
                        nc.vector.tensor_add(t0[:], t0[:], pt[:, 64:128])
                        combine(jq, t0[:, 0:DY], t0[:, DY:2 * DY])

            return emit_stage, emit_z, tail

        zM_nat = const.tile([128, QT * DY], F32, tag="zM_nat")

        def combine_mean(jq, t0, t1):
            sl = slice(jq * DY, (jq + 1) * DY)
            nc.vector.tensor_add(zM_nat[:, sl], t0, t1)
            nc.vector.tensor_scalar_mul(zM_nat[:, sl], zM_nat[:, sl],
                                        e_m[:, jq:jq + 1])

        out_sb = const.tile([128, QT * DY], F32, tag="out_sb")

        def combine_var(jq, t0, t1):
            sl = slice(jq * DY, (jq + 1) * DY)
            nc.vector.tensor_add(out_sb[:, sl], t0, t1)
            nc.vector.tensor_scalar_mul(out_sb[:, sl], out_sb[:, sl],
                                        e_v[:, jq:jq + 1])
            nc.vector.tensor_add(out_sb[:, sl], out_sb[:, sl], zM_nat[:, sl])
            nc.vector.tensor_add(out_sb[:, sl], out_sb[:, sl], ymv_sb[:, sl])

        ph_m = phase_mk(XmT_sb, DM, xmT, lam_m_sb, combine_mean, "m")
        ph_v = phase_mk(XvT_sb, DV, xvT, lam_v_sb, combine_var, "v")

        # ---------------- emission schedule ----------------
        # stage A m: first two sub-blocks up front
        stage_a_sub(invm, Zm_sb, "m", 0)
        lam_stage_in(lam_m_sb, "m", 0)
        stage_a_sub(invm, Zm_sb, "m", 1)
        lam_stage_in(lam_m_sb, "m", 1)

        # var-side setup loads (deprioritized after inv m s0/s1)
        nc.gpsimd.dma_start(XvT_sb[0:DV, :], XvT)
        nc.gpsimd.dma_start(Zv_sb[:], Zv)

        def phase_emit(ph, st):
            emit_stage, emit_z, _ = ph
            emit_stage(st)
            if st - Z_LAG >= 0:
                emit_z(st - Z_LAG)

        # phase m stages 0..7 with stage A m s2/s3 interleaved
        for st in range(0, 4):
            phase_emit(ph_m, st)
        stage_a_sub(invm, Zm_sb, "m", 2)
        lam_stage_in(lam_m_sb, "m", 2)
        for st in range(4, 8):
            phase_emit(ph_m, st)
        stage_a_sub(invm, Zm_sb, "m", 3)
        lam_stage_in(lam_m_sb, "m", 3)

        # var X packs + norm row before phase v needs them
        xvp_sb = xpk.tile([128, NXT * DV], BF16, tag="xpk", name="xvp_sb")
        nc.gpsimd.dma_start(xvp_sb[:], Xvp)
        x_norm_row(XvT_sb, DV, xvp_sb, "v")

        for st in range(8, 12):
            phase_emit(ph_m, st)
        stage_a_sub(invv, Zv_sb, "v", 0)
        lam_stage_in(lam_v_sb, "v", 0)
        for st in range(12, 16):
            phase_emit(ph_m, st)
        stage_a_sub(invv, Zv_sb, "v", 1)
        lam_stage_in(lam_v_sb, "v", 1)
        # trailing z of phase m
        for st in range(NST - Z_LAG, NST):
            ph_m[1](st)
        ph_m[2]()  # tail m

        for st in range(0, 4):
            phase_emit(ph_v, st)
        stage_a_sub(invv, Zv_sb, "v", 2)
        lam_stage_in(lam_v_sb, "v", 2)
        for st in range(4, 8):
            phase_emit(ph_v, st)
        stage_a_sub(invv, Zv_sb, "v", 3)
        lam_stage_in(lam_v_sb, "v", 3)
        for st in range(8, 16):
            phase_emit(ph_v, st)
        for st in range(NST - Z_LAG, NST):
            ph_v[1](st)
        ph_v[2]()  # tail v (writes out_sb)

        nc.scalar.dma_start(out.rearrange("(t p) d -> p t d", p=128),
                            out_sb[:].rearrange("p (t d) -> p t d", d=DY))

    nc.compile()
    return nc


def get_nc():
    global _CACHED_NC
    if _CACHED_NC is None:
        _CACHED_NC = _build_nc()
    return _CACHED_NC


def _host_prep(x_mu, y_eta, y_mean, y_var, X_mean, X_var, Z_mean, Z_var,
               kXXmean_inv, kXXvar_inv):
    """Host prep: transposes / slicing / packing / bf16 casts only."""
    BF = ml_dtypes.bfloat16
    C = np.ascontiguousarray

    def pack_inv(inv):
        # per-core [S_SPLIT, NCH, 128, KTC*RSUB] slabs of inv.T in bf16
        invT = C(inv.T).astype(BF)
        packs = []
        for c in range(NCORES):
            A = invT[:, c * RLOC:(c + 1) * RLOC]
            A = A.reshape(NXT, 128, S_SPLIT, RSUB).transpose(2, 0, 1, 3)
            A = A.reshape(S_SPLIT, NCH, KTC, 128, RSUB).transpose(0, 1, 3, 2, 4)
            packs.append(C(A.reshape(S_SPLIT, NCH, 128, KTC * RSUB)))
        return packs

    invm_p = pack_inv(kXXmean_inv)
    invv_p = pack_inv(kXXvar_inv)

    XmT = C(X_mean.T).astype(BF)
    XvT = C(X_var.T).astype(BF)
    Zm = C(Z_mean.reshape(NXT, 128, DY).transpose(1, 0, 2).reshape(128, -1)
           ).astype(BF)
    Zv = C(Z_var.reshape(NXT, 128, DY).transpose(1, 0, 2).reshape(128, -1)
           ).astype(BF)
    Xmp = C(X_mean.reshape(NXT, 128, DM).transpose(1, 0, 2).reshape(128, -1)
            ).astype(BF)
    Xvp = C(X_var.reshape(NXT, 128, DV).transpose(1, 0, 2).reshape(128, -1)
            ).astype(BF)

    yef = y_eta[::-1]

    in_maps = []
    for c in range(NCORES):
        q = slice(c * QLOC, (c + 1) * QLOC)
        qpk = np.stack([x_mu[q], yef[q], y_mean[q], y_var[q]])  # [4,1024,32]
        qpk = qpk.reshape(4, QT, 128, DY).transpose(2, 0, 1, 3)
        in_maps.append({
            "invm": invm_p[c], "invv": invv_p[c],
            "XmT": XmT, "XvT": XvT, "Zm": Zm, "Zv": Zv,
            "xmuT": C(x_mu[q].T).astype(BF),
            "yefT": C(yef[q].T).astype(BF),
            "ymT": C(y_mean[q].T).astype(BF),
            "yvT": C(y_var[q].T).astype(BF),
            "qpk": C(qpk.reshape(128, -1)).astype(np.float32),
            "Xmp": Xmp, "Xvp": Xvp,
        })
    return in_maps


def kernel(x_mu, y_eta, y_mean, y_var, X_mean, X_var, Z_mean, Z_var,
           kXXmean_inv, kXXvar_inv, _trace=False, _tmpdir=None):
    nc = get_nc()
    in_maps = _host_prep(x_mu, y_eta, y_mean, y_var, X_mean, X_var,
                         Z_mean, Z_var, kXXmean_inv, kXXvar_inv)
    res = run_bass_kernel_spmd(nc, in_maps, core_ids=list(range(NCORES)),
                               trace=_trace, tmpdir=_tmpdir)
    out = np.concatenate([res.results[c]["out"] for c in range(NCORES)], axis=0)
    if _trace:
        kernel._last_results = res
    return out
